# revision 1
# baseline (speedup 1.0000x reference)
"""Causal multi-head attention (B=4, N=2048, D=1024, H=16, dk=dv=64) on 8 Trainium2
NeuronCores.

Sharding: tensor-parallel over heads — core c computes QKV projections and
attention for heads 2c, 2c+1 over the full sequence, then an on-device
AllToAll exchanges attention outputs so each core computes the full output
projection for its 1/8 slice of tokens. Host only transposes x, slices
weights, and concatenates the per-core output slices.

Layouts: projections produce Q^T/K^T (feature-major: 128 partitions = the
core's 2 heads x 64 dims, free = 8192 tokens); V is transposed on the PE into
key-major [V_h0 | 1 | V_h1 | 1] blocks so P@V row sums come from the ones
column. Attention uses the S^T = K^T-tiles' scores layout (keys on
partitions), so P^T feeds P@V directly with no transposes in the inner loop.

Matmuls run in float32r (fp32 rounded to 11 mantissa bits, full PE rate) with
fp32 PSUM accumulation; softmax is exp on the scalar engine (no
max-subtraction: scores are O(1) here) and normalization is deferred to after
P@V via a reciprocal + partition-broadcast multiply.
"""

import os

import numpy as np

DEBUG_PHASE = os.environ.get("KERNEL_DEBUG_PHASE", "")

B, N, D = 4, 2048, 1024
H, DK = 16, 64
NCORES = 8
TOK = B * N                 # 8192 tokens
NT = TOK // 512             # 16 token supertiles for projections
KT = D // 128               # 8 contraction tiles of d_model
TPC = TOK // NCORES         # 1024 tokens per core in the output projection
QT_B = N // 512             # 4 query tiles of 512 per batch

_CACHE = {}
TRACE = False
LAST_EXEC_NS = None
LAST_RESULTS = None


def _build():
    import concourse.tile as tile
    from concourse import bacc, mybir

    F32 = mybir.dt.float32
    F32R = mybir.dt.float32r
    Exp = mybir.ActivationFunctionType.Exp
    mult = mybir.AluOpType.mult

    nc = bacc.Bacc("TRN2", target_bir_lowering=False, debug=False, num_devices=NCORES)

    xT_d = nc.dram_tensor("xT", [D, TOK], F32, kind="ExternalInput")
    wq_d = nc.dram_tensor("wq", [D, 128], F32, kind="ExternalInput")
    wk_d = nc.dram_tensor("wk", [D, 128], F32, kind="ExternalInput")
    wv_d = nc.dram_tensor("wv", [D, 128], F32, kind="ExternalInput")
    bq_d = nc.dram_tensor("bq", [128, 1], F32, kind="ExternalInput")
    bk_d = nc.dram_tensor("bk", [128, 1], F32, kind="ExternalInput")
    bv_d = nc.dram_tensor("bv", [128, 1], F32, kind="ExternalInput")
    wo_d = nc.dram_tensor("wo", [D, D], F32, kind="ExternalInput")
    masks_d = nc.dram_tensor("masks", [128, 4 * 512], F32, kind="ExternalInput")
    ones_d = nc.dram_tensor("onesv", [128, 128], F32, kind="ExternalInput")
    ident_d = nc.dram_tensor("ident", [128, 128], F32, kind="ExternalInput")
    out_d = nc.dram_tensor("out", [TPC, D], F32, kind="ExternalOutput")

    with tile.TileContext(nc) as tc:
        with tc.tile_pool(name="dram", bufs=1, space="DRAM") as dram:
            ot_dram = dram.tile([NCORES, 128, TPC], F32R, name="ot_dram")
            a2a_out = dram.tile([NCORES, 128, TPC], F32R, name="a2a_out")

            with tc.tile_pool(name="big12", bufs=1) as big:
                qt = big.tile([128, TOK], F32R, name="qt")
                kt = big.tile([128, TOK], F32R, name="kt")
                vsb = big.tile([128, 64 * 130], F32R, name="vsb")
                masks = big.tile([128, 4 * 512], F32R, name="masks")

                # ---------- phase 0 + 1: constants and projections ----------
                with (
                    tc.tile_pool(name="wts", bufs=1) as wts,
                    tc.tile_pool(name="xt", bufs=12) as xpool,
                    tc.tile_pool(name="vt", bufs=2) as vtpool,
                    tc.tile_pool(name="ps1", bufs=2, space="PSUM") as ps1,
                ):
                    wq_s = wts.tile([128, KT * 128], F32R, name="wq_s")
                    wk_s = wts.tile([128, KT * 128], F32R, name="wk_s")
                    wv_s = wts.tile([128, KT * 128], F32R, name="wv_s")
                    bq_s = wts.tile([128, 1], F32, name="bq_s")
                    bk_s = wts.tile([128, 1], F32, name="bk_s")
                    bv_s = wts.tile([128, 1], F32, name="bv_s")
                    ident = wts.tile([128, 128], F32, name="ident")

                    for w_s, w_d in ((wq_s, wq_d), (wk_s, wk_d), (wv_s, wv_d)):
                        for kk in range(KT):
                            nc.gpsimd.dma_start(
                                w_s[:, 128 * kk:128 * (kk + 1)],
                                w_d[128 * kk:128 * (kk + 1), :],
                            )
                    nc.sync.dma_start(bq_s[:], bq_d[:])
                    nc.sync.dma_start(bk_s[:], bk_d[:])
                    nc.sync.dma_start(bv_s[:], bv_d[:])
                    nc.sync.dma_start(ident[:], ident_d[:])
                    nc.gpsimd.dma_start(masks[:], masks_d[:])
                    # ones columns of vsb at free index 65*j + 64, j = 0..127
                    ones_sb = wts.tile([128, 128], F32, name="ones_sb")
                    nc.sync.dma_start(ones_sb[:], ones_d[:])
                    nc.vector.tensor_copy(
                        vsb[:].rearrange("p (j c) -> p j c", c=65)[:, :, 64:65],
                        ones_sb[:].rearrange("p (j c) -> p j c", c=1),
                    )

                    for tt in range(NT):
                        qt_ps = ps1.tile([128, 512], F32, name="qt_ps")
                        kt_ps = ps1.tile([128, 512], F32, name="kt_ps")
                        vt_ps = ps1.tile([128, 512], F32, name="vt_ps")
                        for kk in range(KT):
                            xt = xpool.tile([128, 512], F32R, name="xt")
                            nc.gpsimd.dma_start(
                                xt[:],
                                xT_d[128 * kk:128 * (kk + 1), 512 * tt:512 * (tt + 1)],
                            )
                            f, l = kk == 0, kk == KT - 1
                            ksl = slice(128 * kk, 128 * (kk + 1))
                            nc.tensor.matmul(qt_ps[:], wq_s[:, ksl], xt[:], start=f, stop=l)
                            nc.tensor.matmul(kt_ps[:], wk_s[:, ksl], xt[:], start=f, stop=l)
                            nc.tensor.matmul(vt_ps[:], wv_s[:, ksl], xt[:], start=f, stop=l)
                        sl = slice(512 * tt, 512 * (tt + 1))
                        nc.vector.tensor_scalar_add(qt[:, sl], qt_ps[:], bq_s[:])
                        nc.vector.tensor_scalar_add(kt[:, sl], kt_ps[:], bk_s[:])
                        vt_sb = vtpool.tile([128, 512], F32, name="vt_sb")
                        nc.vector.tensor_scalar_add(vt_sb[:], vt_ps[:], bv_s[:])
                        for j in range(4):
                            vtr_ps = ps1.tile([128, 128], F32, name="vtr_ps")
                            nc.tensor.transpose(
                                vtr_ps[:], vt_sb[:, 128 * j:128 * (j + 1)], ident[:]
                            )
                            base = (4 * tt + j) * 130
                            nc.vector.tensor_copy(
                                vsb[:, base:base + 130]
                                .rearrange("p (h c) -> p h c", h=2)[:, :, 0:64],
                                vtr_ps[:].rearrange("p (h c) -> p h c", h=2),
                            )

                # ---------- phase 2: attention ----------
                if DEBUG_PHASE != "1":
                    with (
                        tc.tile_pool(name="ot", bufs=2) as otp,
                        tc.tile_pool(name="pt", bufs=4) as ptp,
                        tc.tile_pool(name="bc", bufs=2) as bcp,
                        tc.tile_pool(name="rc", bufs=2) as rcp,
                        tc.tile_pool(name="sps", bufs=2, space="PSUM") as sps,
                        tc.tile_pool(name="ops", bufs=2, space="PSUM") as ops,
                    ):
                        for b in range(B):
                            tb = N * b
                            oth0 = otp.tile([64, N], F32R, name="oth0", padded_shape=[128, N])
                            oth1 = otp.tile([64, N], F32R, name="oth1", padded_shape=[128, N])
                            for qq in range(QT_B):
                                qsl = slice(tb + 512 * qq, tb + 512 * (qq + 1))
                                o_ps0 = ops.tile([65, 512], F32, name="o_ps0")
                                o_ps1 = ops.tile([65, 512], F32, name="o_ps1")
                                kmax = 4 * qq + 3
                                # software pipeline: emit scores for kk, P@V for
                                # kk-1, so the in-order PE never stalls on exp(kk)
                                def pv(pp0, pp1, pkk, last):
                                    vb = (16 * b + pkk) * 130
                                    f = pkk == 0
                                    nc.tensor.matmul(
                                        o_ps0[:], vsb[:, vb:vb + 65], pp0[:],
                                        start=f, stop=last,
                                    )
                                    nc.tensor.matmul(
                                        o_ps1[:], vsb[:, vb + 65:vb + 130], pp1[:],
                                        start=f, stop=last,
                                    )
                                pend = None
                                for kk in range(kmax + 1):
                                    ksl = slice(tb + 128 * kk, tb + 128 * (kk + 1))
                                    s_ps0 = sps.tile([128, 512], F32, name="s_ps0")
                                    s_ps1 = sps.tile([128, 512], F32, name="s_ps1")
                                    nc.tensor.matmul(
                                        s_ps0[:], kt[0:64, ksl], qt[0:64, qsl],
                                        start=True, stop=True,
                                    )
                                    nc.tensor.matmul(
                                        s_ps1[:], kt[64:128, ksl], qt[64:128, qsl],
                                        start=True, stop=True, tile_position=(64, 0),
                                    )
                                    p0 = ptp.tile([128, 512], F32R, name="p0")
                                    p1 = ptp.tile([128, 512], F32R, name="p1")
                                    nc.scalar.activation(p0[:], s_ps0[:], Exp, scale=0.125)
                                    nc.scalar.activation(p1[:], s_ps1[:], Exp, scale=0.125)
                                    r = kk - 4 * qq
                                    if r >= 0:  # diagonal tile: apply causal mask
                                        msl = slice(512 * r, 512 * (r + 1))
                                        nc.vector.tensor_tensor(p0[:], p0[:], masks[:, msl], op=mult)
                                        nc.vector.tensor_tensor(p1[:], p1[:], masks[:, msl], op=mult)
                                    if pend is not None:
                                        pv(*pend, last=False)
                                    pend = (p0, p1, kk)
                                pv(*pend, last=True)
                                # normalize by the ones-column row sums
                                osl = slice(512 * qq, 512 * (qq + 1))
                                for o_ps, oth in ((o_ps0, oth0), (o_ps1, oth1)):
                                    rc = rcp.tile([1, 512], F32, name="rc")
                                    nc.vector.reciprocal(rc[:], o_ps[64:65, :])
                                    bc = bcp.tile([64, 512], F32, name="bc", padded_shape=[128, 512])
                                    nc.gpsimd.partition_broadcast(bc[:], rc[0:1, :])
                                    nc.vector.tensor_tensor(
                                        oth[:, osl], o_ps[0:64, :], bc[:], op=mult
                                    )
                            # stage this batch's O^T out to DRAM in AllToAll chunk layout
                            for half in range(2):
                                j = 2 * b + half
                                csl = slice(TPC * half, TPC * (half + 1))
                                nc.sync.dma_start(ot_dram[j, 0:64, :], oth0[:, csl])
                                nc.sync.dma_start(ot_dram[j, 64:128, :], oth1[:, csl])

            # ---------- AllToAll: heads-major -> token-sliced ----------
            if DEBUG_PHASE == "2":
                nc.sync.dma_start(
                    out_d[:],
                    ot_dram[:].rearrange("a p m -> (a p) m").bitcast(F32),
                )
            if DEBUG_PHASE not in ("1", "2"):
                nc.gpsimd.collective_compute(
                    "AllToAll",
                    mybir.AluOpType.bypass,
                    replica_groups=[list(range(NCORES))],
                    ins=[ot_dram[:]],
                    outs=[a2a_out[:]],
                )

                # ---------- phase 3: output projection for this core's tokens ----------
                with (
                    tc.tile_pool(name="wo3", bufs=1) as wop,
                    tc.tile_pool(name="ot3", bufs=1) as ot3,
                    tc.tile_pool(name="os3", bufs=2) as os3,
                    tc.tile_pool(name="ps3", bufs=2, space="PSUM") as ps3,
                ):
                    wo_s = wop.tile([128, KT * D], F32R, name="wo_s")
                    for kk in range(KT):
                        nc.gpsimd.dma_start(
                            wo_s[:, D * kk:D * (kk + 1)],
                            wo_d[128 * kk:128 * (kk + 1), :],
                        )
                    ot_t = ot3.tile([128, KT * TPC], F32R, name="ot_t")
                    for kk in range(KT):
                        nc.sync.dma_start(
                            ot_t[:, TPC * kk:TPC * (kk + 1)], a2a_out[kk]
                        )
                    for j in range(TPC // 128):
                        out_ps0 = ps3.tile([128, 512], F32, name="out_ps0")
                        out_ps1 = ps3.tile([128, 512], F32, name="out_ps1")
                        for kk in range(KT):
                            f, l = kk == 0, kk == KT - 1
                            lhs = ot_t[:, kk * TPC + 128 * j: kk * TPC + 128 * (j + 1)]
                            nc.tensor.matmul(
                                out_ps0[:], lhs, wo_s[:, kk * D:kk * D + 512],
                                start=f, stop=l,
                            )
                            nc.tensor.matmul(
                                out_ps1[:], lhs, wo_s[:, kk * D + 512:kk * D + 1024],
                                start=f, stop=l,
                            )
                        out_sb = os3.tile([128, D], F32, name="out_sb")
                        nc.vector.tensor_copy(out_sb[:, 0:512], out_ps0[:])
                        nc.vector.tensor_copy(out_sb[:, 512:1024], out_ps1[:])
                        nc.sync.dma_start(out_d[128 * j:128 * (j + 1), :], out_sb[:])

    nc.compile()
    return nc


def _host_prep(inputs):
    x = np.asarray(inputs["x"], np.float32)
    Wq = np.asarray(inputs["Wq"], np.float32)
    bq = np.asarray(inputs["bq"], np.float32)
    Wk = np.asarray(inputs["Wk"], np.float32)
    bk = np.asarray(inputs["bk"], np.float32)
    Wv = np.asarray(inputs["Wv"], np.float32)
    bv = np.asarray(inputs["bv"], np.float32)
    Wo = np.asarray(inputs["Wo"], np.float32)

    xT = np.ascontiguousarray(x.reshape(TOK, D).T)
    woT = np.ascontiguousarray(Wo.T)
    ident = np.eye(128, dtype=np.float32)
    onesv = np.ones((128, 128), np.float32)
    masks = np.zeros((128, 4, 512), np.float32)
    k_idx = np.arange(128)[:, None]
    q_idx = np.arange(512)[None, :]
    for r in range(4):
        masks[:, r, :] = (q_idx >= 128 * r + k_idx).astype(np.float32)
    masks = masks.reshape(128, 4 * 512)

    in_maps = []
    for c in range(NCORES):
        sl = slice(128 * c, 128 * (c + 1))
        in_maps.append({
            "xT": xT,
            "wq": np.ascontiguousarray(Wq[sl].T),
            "wk": np.ascontiguousarray(Wk[sl].T),
            "wv": np.ascontiguousarray(Wv[sl].T),
            "bq": np.ascontiguousarray(bq[sl].reshape(128, 1)),
            "bk": np.ascontiguousarray(bk[sl].reshape(128, 1)),
            "bv": np.ascontiguousarray(bv[sl].reshape(128, 1)),
            "wo": woT,
            "masks": masks,
            "onesv": onesv,
            "ident": ident,
        })
    return in_maps


def kernel(**inputs):
    global LAST_EXEC_NS, LAST_RESULTS
    from concourse.bass_utils import run_bass_kernel_spmd

    if "nc" not in _CACHE:
        _CACHE["nc"] = _build()
    nc = _CACHE["nc"]
    in_maps = _host_prep(inputs)
    res = run_bass_kernel_spmd(nc, in_maps, list(range(NCORES)), trace=TRACE)
    LAST_EXEC_NS = res.exec_time_ns
    LAST_RESULTS = res
    out = np.concatenate([res.results[c]["out"] for c in range(NCORES)], axis=0)
    return out.reshape(B, N, D)



# revision 6
# speedup vs baseline: 1.2850x; 1.2850x over previous
"""Causal multi-head attention (B=4, N=2048, D=1024, H=16, dk=dv=64) on 8 Trainium2
NeuronCores.

Sharding: tensor-parallel over heads — core c computes QKV projections and
attention for heads 2c, 2c+1. v2 restructures the baseline into a
batch-pipelined schedule: per batch b we emit QKV(b) -> attention(b) ->
staged per-batch AllToAll(b), with output projection of batch b-1 emitted in
the middle of attention(b). Tile's list scheduler then interleaves QKV
matmuls of the next batch into attention's exp-wait gaps, keeping the PE
dense (HAM stays at full clock) and hiding the collectives.

Attention uses the S^T layout (keys on partitions). Per (q-tile, key-tile)
iteration both heads' scores go into adjacent PSUM banks so ONE activation
instruction computes exp for both heads; diagonal band tiles use trapezoid
free dims (only valid queries) plus a single [128,128] triangle mask. P and
V are bf16 (fp32 PSUM accumulation); softmax denominators come from a ones
column appended to V. Normalization is deferred to after P@V.
"""

import numpy as np

B, N, D = 4, 2048, 1024
NCORES = 8
KT = D // 128            # 8 contraction tiles of d_model
TPB = N                  # 2048 tokens per batch
NT_B = TPB // 512        # 4 projection supertiles per batch
QT_B = TPB // 512        # 4 query tiles per batch
EV = TPB // NCORES       # 256 tokens per core per A2A event

_CACHE = {}
TRACE = False
LAST_EXEC_NS = None
LAST_RESULTS = None


def _build():
    import concourse.tile as tile
    from concourse import bacc, mybir

    F32 = mybir.dt.float32
    F32R = mybir.dt.float32r
    BF16 = mybir.dt.bfloat16
    Exp = mybir.ActivationFunctionType.Exp
    mult = mybir.AluOpType.mult

    nc = bacc.Bacc("TRN2", target_bir_lowering=False, debug=False, num_devices=NCORES)

    xT_d = nc.dram_tensor("xT", [D, B * N], F32, kind="ExternalInput")
    wq_d = nc.dram_tensor("wq", [D, 128], F32, kind="ExternalInput")
    wk_d = nc.dram_tensor("wk", [D, 128], F32, kind="ExternalInput")
    wv_d = nc.dram_tensor("wv", [D, 128], F32, kind="ExternalInput")
    bq_d = nc.dram_tensor("bq", [128, 1], F32, kind="ExternalInput")
    bk_d = nc.dram_tensor("bk", [128, 1], F32, kind="ExternalInput")
    bv_d = nc.dram_tensor("bv", [128, 1], F32, kind="ExternalInput")
    wo_d = nc.dram_tensor("wo", [D, D], F32, kind="ExternalInput")
    ident_d = nc.dram_tensor("ident", [128, 128], F32, kind="ExternalInput")
    tri_d = nc.dram_tensor("tri", [128, 128], F32, kind="ExternalInput")
    out_d = nc.dram_tensor("out", [B * EV, D], F32, kind="ExternalOutput")

    with tile.TileContext(nc) as tc:
        with (
            tc.tile_pool(name="dram", bufs=1, space="DRAM") as dram,
            tc.tile_pool(name="wts", bufs=1) as wts,
            tc.tile_pool(name="xp", bufs=12) as xp,
            tc.tile_pool(name="qkvp", bufs=2) as qkvp,
            tc.tile_pool(name="vtp", bufs=2) as vtp,
            tc.tile_pool(name="pbp", bufs=4) as pbp,
            tc.tile_pool(name="othp", bufs=2) as othp,
            tc.tile_pool(name="nrmp", bufs=2) as nrmp,
            tc.tile_pool(name="otp", bufs=2) as otp,
            tc.tile_pool(name="osbp", bufs=2) as osbp,
            # PSUM: scores 2x[128,1024] = 4 banks, PV accums = 2 banks,
            # misc (QKV/V-transpose/outproj) 2x[128,512] = 2 banks -> 8 total
            tc.tile_pool(name="scps", bufs=2, space="PSUM") as scps,
            tc.tile_pool(name="pvps", bufs=1, space="PSUM") as pvps,
            tc.tile_pool(name="mps", bufs=2, space="PSUM") as mps,
        ):
            # ---------------- prologue: constants and weights ----------------
            wq_s = wts.tile([128, KT * 128], F32R, name="wq_s")
            wk_s = wts.tile([128, KT * 128], F32R, name="wk_s")
            wv_s = wts.tile([128, KT * 128], F32R, name="wv_s")
            bq_s = wts.tile([128, 1], F32, name="bq_s")
            bk_s = wts.tile([128, 1], F32, name="bk_s")
            bv_s = wts.tile([128, 1], F32, name="bv_s")
            ident_f = wts.tile([128, 128], F32, name="ident_f")
            tri_f = wts.tile([128, 128], F32, name="tri_f")
            ident_b = wts.tile([128, 128], BF16, name="ident_b")
            tri_b = wts.tile([128, 128], BF16, name="tri_b")
            wo_s = wts.tile([128, KT * D], F32R, name="wo_s")

            for w_s, w_d, eng in (
                (wq_s, wq_d, nc.sync),
                (wk_s, wk_d, nc.gpsimd),
                (wv_s, wv_d, nc.sync),
            ):
                for kk in range(KT):
                    eng.dma_start(
                        w_s[:, 128 * kk:128 * (kk + 1)],
                        w_d[128 * kk:128 * (kk + 1), :].bitcast(F32R),
                    )
            nc.sync.dma_start(bq_s[:], bq_d[:])
            nc.sync.dma_start(bk_s[:], bk_d[:])
            nc.sync.dma_start(bv_s[:], bv_d[:])
            nc.sync.dma_start(ident_f[:], ident_d[:])
            nc.sync.dma_start(tri_f[:], tri_d[:])
            nc.vector.tensor_copy(ident_b[:], ident_f[:])
            nc.vector.tensor_copy(tri_b[:], tri_f[:])

            ot_dram = [
                dram.tile([NCORES, 128, EV], F32R, name=f"otd{b}") for b in range(B)
            ]
            a2a = [
                dram.tile([NCORES, 128, EV], F32R, name=f"a2a{b}") for b in range(B)
            ]

            def load_wo():
                # Wo^T, feat-major chunks; split across two queues, emitted
                # after batch-0's x tiles so it never delays QKV(0).
                for kk in range(KT):
                    eng = nc.sync if kk % 2 == 0 else nc.gpsimd
                    eng.dma_start(
                        wo_s[:, D * kk:D * (kk + 1)],
                        wo_d[128 * kk:128 * (kk + 1), :].bitcast(F32R),
                    )

            def qkv_batch(b):
                qt = qkvp.tile([128, TPB], F32R, name="qt_b")
                kt = qkvp.tile([128, TPB], F32R, name="kt_b")
                vsb = qkvp.tile([128, 16 * 130], BF16, name="vsb_b")
                # ones columns for the softmax denominator, free index 65j+64
                nc.vector.memset(
                    vsb[:].rearrange("p (j c) -> p j c", c=65)[:, :, 64:65], 1.0
                )
                for tt in range(NT_B):
                    xts = []
                    for kk in range(KT):
                        xt = xp.tile([128, 512], F32R, name="xt")
                        nc.gpsimd.dma_start(
                            xt[:],
                            xT_d[
                                128 * kk:128 * (kk + 1),
                                N * b + 512 * tt:N * b + 512 * (tt + 1),
                            ],
                        )
                        xts.append(xt)
                    sl = slice(512 * tt, 512 * (tt + 1))
                    # q/k/v sequentially so only one PSUM accumulator is live
                    q_ps = mps.tile([128, 512], F32, name="q_ps", tag="mx")
                    for kk in range(KT):
                        nc.tensor.matmul(
                            q_ps[:], wq_s[:, 128 * kk:128 * (kk + 1)], xts[kk][:],
                            start=kk == 0, stop=kk == KT - 1,
                        )
                    nc.vector.tensor_scalar_add(qt[:, sl], q_ps[:], bq_s[:])
                    k_ps = mps.tile([128, 512], F32, name="k_ps", tag="mx")
                    for kk in range(KT):
                        nc.tensor.matmul(
                            k_ps[:], wk_s[:, 128 * kk:128 * (kk + 1)], xts[kk][:],
                            start=kk == 0, stop=kk == KT - 1,
                        )
                    nc.vector.tensor_scalar_add(kt[:, sl], k_ps[:], bk_s[:])
                    v_ps = mps.tile([128, 512], F32, name="v_ps", tag="mx")
                    for kk in range(KT):
                        nc.tensor.matmul(
                            v_ps[:], wv_s[:, 128 * kk:128 * (kk + 1)], xts[kk][:],
                            start=kk == 0, stop=kk == KT - 1,
                        )
                    vt_sb = vtp.tile([128, 512], BF16, name="vt_sb")
                    nc.vector.tensor_scalar_add(vt_sb[:], v_ps[:], bv_s[:])
                    # transpose V to key-major and interleave [Vh0 | 1 | Vh1 | 1]
                    for j in range(4):
                        vtr = mps.tile([128, 128], BF16, name="vtr", tag="mx")
                        nc.tensor.transpose(
                            vtr[:], vt_sb[:, 128 * j:128 * (j + 1)], ident_b[:]
                        )
                        base = (4 * tt + j) * 130
                        nc.vector.tensor_copy(
                            vsb[:, base:base + 130]
                            .rearrange("p (h c) -> p h c", h=2)[:, :, 0:64],
                            vtr[:].rearrange("p (h c) -> p h c", h=2),
                        )
                return qt, kt, vsb

            def outproj_batch(b):
                ot_t = otp.tile([128, NCORES * EV], F32R, name="ot_t")
                for s in range(NCORES):
                    nc.sync.dma_start(ot_t[:, EV * s:EV * (s + 1)], a2a[b][s])
                for j in range(EV // 128):
                    osb = osbp.tile([128, D], F32, name="osb")
                    for half in range(2):
                        ps = mps.tile([128, 512], F32, name="op_ps", tag="mx")
                        for s in range(NCORES):
                            lhs = ot_t[:, EV * s + 128 * j:EV * s + 128 * (j + 1)]
                            nc.tensor.matmul(
                                ps[:], lhs, wo_s[:, D * s + 512 * half:D * s + 512 * (half + 1)],
                                start=s == 0, stop=s == NCORES - 1,
                            )
                        nc.vector.tensor_copy(osb[:, 512 * half:512 * (half + 1)], ps[:])
                    nc.sync.dma_start(
                        out_d[EV * b + 128 * j:EV * b + 128 * (j + 1), :], osb[:]
                    )

            def attention_batch(b, qt, kt, vsb, oth0, oth1, mid=None):
                for qq in range(QT_B):
                    if qq == 3 and mid is not None:
                        mid()
                    o_ps0 = pvps.tile([65, 512], F32, name="o_ps0")
                    o_ps1 = pvps.tile([65, 512], F32, name="o_ps1")
                    kmax = 4 * qq + 3

                    def pv(p_both, pkk, cols, qoff, last):
                        vb = 130 * pkk
                        f = pkk == 0
                        nc.tensor.matmul(
                            o_ps0[:, qoff:qoff + cols], vsb[:, vb:vb + 65],
                            p_both[:, 0:cols],
                            start=f, stop=last, skip_group_check=True,
                        )
                        nc.tensor.matmul(
                            o_ps1[:, qoff:qoff + cols], vsb[:, vb + 65:vb + 130],
                            p_both[:, 512:512 + cols],
                            start=f, stop=last, skip_group_check=True,
                        )

                    pend = None
                    for kk in range(kmax + 1):
                        r = kk - 4 * qq
                        cols = 512 if r < 0 else 512 - 128 * r
                        qoff = 0 if r < 0 else 128 * r
                        qsl = slice(512 * qq + qoff, 512 * (qq + 1))
                        ksl = slice(128 * kk, 128 * (kk + 1))
                        s_both = scps.tile([128, 1024], F32, name="s_both")
                        nc.tensor.matmul(
                            s_both[:, 0:cols], kt[0:64, ksl], qt[0:64, qsl],
                            start=True, stop=True,
                        )
                        nc.tensor.matmul(
                            s_both[:, 512:512 + cols], kt[64:128, ksl], qt[64:128, qsl],
                            start=True, stop=True, tile_position=(64, 0),
                        )
                        p_both = pbp.tile([128, 1024], BF16, name="p_both")
                        sv = s_both[:].rearrange("p (h c) -> p h c", c=512)[:, :, 0:cols]
                        pw = p_both[:].rearrange("p (h c) -> p h c", c=512)[:, :, 0:cols]
                        nc.scalar.activation(pw, sv, Exp, scale=0.125)
                        if r >= 0:
                            # first 128 columns of each head's region are the
                            # diagonal triangle
                            nc.vector.tensor_tensor(
                                p_both[:, 0:128], p_both[:, 0:128], tri_b[:], op=mult
                            )
                            nc.vector.tensor_tensor(
                                p_both[:, 512:640], p_both[:, 512:640], tri_b[:], op=mult
                            )
                        if pend is not None:
                            pv(*pend, last=False)
                        pend = (p_both, kk, cols, qoff)
                    pv(*pend, last=True)
                    # normalize by the ones-column row sums
                    osl = slice(512 * qq, 512 * (qq + 1))
                    for o_ps, oth in ((o_ps0, oth0), (o_ps1, oth1)):
                        rc = nrmp.tile([1, 512], F32, name="rc")
                        nc.vector.reciprocal(rc[:], o_ps[64:65, :])
                        bc = nrmp.tile([64, 512], F32, name="bc", padded_shape=[128, 512])
                        nc.gpsimd.partition_broadcast(bc[:], rc[0:1, :])
                        nc.vector.tensor_tensor(
                            oth[:, osl], o_ps[0:64, :], bc[:], op=mult
                        )
                    # stage this q-tile's chunks for the per-batch AllToAll
                    for i in (2 * qq, 2 * qq + 1):
                        csl = slice(EV * i, EV * (i + 1))
                        nc.sync.dma_start(ot_dram[b][i, 0:64, :], oth0[:, csl])
                        nc.sync.dma_start(ot_dram[b][i, 64:128, :], oth1[:, csl])

            for b in range(B):
                qt, kt, vsb = qkv_batch(b)
                if b == 0:
                    load_wo()
                oth0 = othp.tile([64, TPB], F32R, name="oth0", padded_shape=[128, TPB])
                oth1 = othp.tile([64, TPB], F32R, name="oth1", padded_shape=[128, TPB])
                mid = (lambda bb=b: outproj_batch(bb - 1)) if b >= 1 else None
                attention_batch(b, qt, kt, vsb, oth0, oth1, mid=mid)
                nc.gpsimd.collective_compute(
                    "AllToAll",
                    mybir.AluOpType.bypass,
                    replica_groups=[list(range(NCORES))],
                    ins=[ot_dram[b][:]],
                    outs=[a2a[b][:]],
                )
            outproj_batch(B - 1)

    nc.compile()
    return nc


def _host_prep(inputs):
    x = np.asarray(inputs["x"], np.float32)
    Wq = np.asarray(inputs["Wq"], np.float32)
    bq = np.asarray(inputs["bq"], np.float32)
    Wk = np.asarray(inputs["Wk"], np.float32)
    bk = np.asarray(inputs["bk"], np.float32)
    Wv = np.asarray(inputs["Wv"], np.float32)
    bv = np.asarray(inputs["bv"], np.float32)
    Wo = np.asarray(inputs["Wo"], np.float32)

    xT = np.ascontiguousarray(x.reshape(B * N, D).T)
    woT = np.ascontiguousarray(Wo.T)
    ident = np.eye(128, dtype=np.float32)
    k_idx = np.arange(128)[:, None]
    j_idx = np.arange(128)[None, :]
    tri = (j_idx >= k_idx).astype(np.float32)

    in_maps = []
    for c in range(NCORES):
        sl = slice(128 * c, 128 * (c + 1))
        in_maps.append({
            "xT": xT,
            "wq": np.ascontiguousarray(Wq[sl].T),
            "wk": np.ascontiguousarray(Wk[sl].T),
            "wv": np.ascontiguousarray(Wv[sl].T),
            "bq": np.ascontiguousarray(bq[sl].reshape(128, 1)),
            "bk": np.ascontiguousarray(bk[sl].reshape(128, 1)),
            "bv": np.ascontiguousarray(bv[sl].reshape(128, 1)),
            "wo": woT,
            "ident": ident,
            "tri": tri,
        })
    return in_maps


def kernel(**inputs):
    global LAST_EXEC_NS, LAST_RESULTS
    from concourse.bass_utils import run_bass_kernel_spmd

    if "nc" not in _CACHE:
        _CACHE["nc"] = _build()
    nc = _CACHE["nc"]
    in_maps = _host_prep(inputs)
    res = run_bass_kernel_spmd(nc, in_maps, list(range(NCORES)), trace=TRACE)
    LAST_EXEC_NS = res.exec_time_ns
    LAST_RESULTS = res
    out = np.empty((B, N, D), np.float32)
    for c in range(NCORES):
        co = res.results[c]["out"].reshape(B, EV, D)
        out[:, EV * c:EV * (c + 1), :] = co
    return out
